# revision 16
# baseline (speedup 1.0000x reference)
"""DistMult scoring kernel for Trainium2 (8 NeuronCores, SPMD batch-parallel).

score = sigmoid(sum_d ent[h]_d * rel[r]_d * ent[t]_d)

The axon tunnel to the devices moves ~30-85 MB/s, so per-call host->device
bytes dominate end-to-end time. The 512 MB ent table is therefore shipped
ROW-SHARDED in fp16 (33.5 MB per core, 268 MB total instead of 4.2 GB
replicated fp32) and re-assembled on device with a DRAM AllGather over
NeuronLink before the gather phase. Everything else is identical to the
replicated design:

- 1,048,576 triples split across 8 cores (131,072 each); rel_emb replicated.
- ent rows fetched from the AllGather result with [P,1] indirect DMAs
  (the HW consumes exactly one index per partition per indirect DMA
  command): 128 rows x 256 B per instruction.
- rel rows (< 500, fits int16) fetched with dma_gather: indices shipped as
  a [16, COLS*8] int16 block and replicated to 128 partitions on device.
- Gather completion detected with a flush barrier: a tiny normal SWDGE DMA
  on the same qPoolDynamic queue lands after every prior gather descriptor
  and bumps its semaphore by exactly 16. (The increments attached to the
  gather instructions themselves fire early on HW — do not gate on them.)
- ACT upconverts the fp16 rows to fp32 (exact), DVE computes h*t*r and a
  segmented 128-wide reduction, ACT applies the sigmoid, one full-rate DMA
  writes the fp32 scores out.

fp16 quantization of h/t gives max rel err ~1.2e-2 on the pre-sigmoid sum
tails (gate 2e-2); set TABLE_FP16 = False to ship fp32 shards (524 MB).
"""
import os

os.environ.setdefault("NEURON_RT_RESET_CORES", "1")

import numpy as np
import concourse.bacc as bacc
import concourse.bass as bass
from concourse import mybir
from concourse.bass_utils import run_bass_kernel_spmd

N_CORES = 8
P, D = 128, 128
B = 1_048_576
B_CORE = B // N_CORES            # 131072 triples per core
COLS = B_CORE // P               # 1024 triples per partition
K = 8                            # columns per super-tile (1024 triples)
N_SUPER = COLS // K
ENT = 1_000_000
SHARD_FULL = 131072              # full-table shard rows (8x = 1048576 >= ENT)
# ~12% of rows are never referenced by the 2M random draws; ship only the
# referenced ones (877,342 for the reference seed) remapped to compact ids.
SHARD_COMPACT = 110592           # 8x = 884736 capacity; fallback: full table
REL = 500
N_BUFS = 2
N_QUEUES = 4     # SWDGE queues; each is serviced by its own Q7 core pair
TABLE_FP16 = True

# flat int32 layout of the packed small-input tensor
OFF_BH = 0
OFF_BT = OFF_BH + P * COLS                 # 131072
OFF_R16 = OFF_BT + P * COLS                # 262144
OFF_REL = OFF_R16 + (16 * COLS * 8) // 2   # 327680 (int16 pairs)
AUX_WORDS = OFF_REL + REL * D              # 391680 (f32 words)

_CACHED_NC = {}                  # shard_rows -> compiled nc
_MODE_SHARD = SHARD_COMPACT      # set by make_in_maps, read by _get_nc


def _set_queue(inst, q):
    if q:
        inst.ins.queue = f"qPoolDynamic{q}"
    return inst


def _build_nc(shard_rows):
    cols, k, n_bufs, n_queues = COLS, K, N_BUFS, N_QUEUES
    SHARD, ENT_PAD = shard_rows, N_CORES * shard_rows
    assert cols % k == 0 and k % n_queues == 0
    n_super = cols // k
    tdt = mybir.dt.float16 if TABLE_FP16 else mybir.dt.float32
    nc = bacc.Bacc(num_swdge_queues=n_queues)
    # all small inputs ride in ONE flat int32 tensor (each separate jit arg
    # costs ~0.12 s of fixed tunnel overhead): bh | bt | r16(int16) | rel(f32)
    aux = nc.dram_tensor("aux", [AUX_WORDS], mybir.dt.int32,
                         kind="ExternalInput")
    ent_shard = nc.dram_tensor("ent_shard", [SHARD, D], tdt, kind="ExternalInput")
    bh = aux[OFF_BH:OFF_BT].rearrange("(p c) -> p c", c=cols)
    bt = aux[OFF_BT:OFF_R16].rearrange("(p c) -> p c", c=cols)
    br = aux[OFF_R16:OFF_REL].bitcast(mybir.dt.int16).rearrange(
        "(p c) -> p c", c=cols * 8)
    rel = aux[OFF_REL:AUX_WORDS].bitcast(mybir.dt.float32).rearrange(
        "(r d) -> r d", d=D)
    score = nc.dram_tensor("score", [P, cols], mybir.dt.float32,
                           kind="ExternalOutput")
    # collectives can't touch I/O tensors: bounce the shard, gather to DRAM.
    # Shared addr space: the 8 cores sit on one chip's HBM, so a Shared
    # AllGather output is written once instead of ring-replicated per core.
    cc_in = nc.dram_tensor("cc_in", [SHARD, D], tdt)
    ent_full = nc.dram_tensor("ent_full", [ENT_PAD, D], tdt, addr_space="Shared")

    n_idx = 128 * k

    from contextlib import ExitStack

    with ExitStack() as stack:
        ec = stack.enter_context
        h_idx = ec(nc.sbuf_tensor("h_idx", [P, cols], mybir.dt.int32))
        t_idx = ec(nc.sbuf_tensor("t_idx", [P, cols], mybir.dt.int32))
        r_idx = ec(nc.sbuf_tensor("r_idx", [128, cols * 8], mybir.dt.int16))
        scores = ec(nc.sbuf_tensor("scores", [P, cols], mybir.dt.float32))
        sig = ec(nc.sbuf_tensor("sig", [P, cols], mybir.dt.float32))
        flush_a = ec(nc.sbuf_tensor("flush_a", [P, n_queues], mybir.dt.float32))
        flush_b = ec(nc.sbuf_tensor("flush_b", [P, n_queues], mybir.dt.float32))
        h16_buf = ec(nc.sbuf_tensor("h16_buf", [P, n_bufs * k * D], tdt))
        t16_buf = ec(nc.sbuf_tensor("t16_buf", [P, n_bufs * k * D], tdt))
        h_buf = ec(nc.sbuf_tensor("h_buf", [P, n_bufs * k * D], mybir.dt.float32))
        t_buf = ec(nc.sbuf_tensor("t_buf", [P, n_bufs * k * D], mybir.dt.float32))
        r_buf = ec(nc.sbuf_tensor("r_buf", [P, n_bufs * k * D], mybir.dt.float32))
        i_sem = ec(nc.semaphore("i_sem"))
        r_sem = ec(nc.semaphore("r_sem"))
        b_sem = ec(nc.semaphore("b_sem"))
        cc_sem = ec(nc.semaphore("cc_sem"))
        gh_sem = ec(nc.semaphore("gh_sem"))
        gt_sem = ec(nc.semaphore("gt_sem"))
        gr_sem = ec(nc.semaphore("gr_sem"))
        f_sem = ec(nc.semaphore("f_sem"))
        c_sem = ec(nc.semaphore("c_sem"))
        v_sem = ec(nc.semaphore("v_sem"))
        s_sem = ec(nc.semaphore("s_sem"))
        o_sem = ec(nc.semaphore("o_sem"))
        block = ec(nc.Block())
        def bufsl(buf, s, j=None):
            b = s % n_bufs
            if j is None:
                return buf[:, b * k * D:(b + 1) * k * D]
            return buf[:, (b * k + j) * D:(b * k + j + 1) * D]

        @block.sync
        def _(sync):
            sync.dma_start(out=h_idx[:], in_=bh).then_inc(i_sem, 16)
            sync.dma_start(out=t_idx[:], in_=bt).then_inc(i_sem, 16)
            # rel indices arrive as 16 partitions; double up to 128 on device
            sync.dma_start(out=r_idx[0:16, :], in_=br).then_inc(r_sem, 16)
            sync.wait_ge(r_sem, 16)
            sync.dma_start(out=r_idx[16:32, :], in_=r_idx[0:16, :]).then_inc(r_sem, 16)
            sync.wait_ge(r_sem, 32)
            sync.dma_start(out=r_idx[32:64, :], in_=r_idx[0:32, :]).then_inc(r_sem, 16)
            sync.wait_ge(r_sem, 48)
            sync.dma_start(out=r_idx[64:128, :], in_=r_idx[0:64, :]).then_inc(r_sem, 16)
            sync.wait_ge(s_sem, 1)
            sync.dma_start(out=score[:], in_=sig[:]).then_inc(o_sem, 16)

        @block.gpsimd
        def _(g):
            g.dma_start(out=cc_in[:], in_=ent_shard[:]).then_inc(b_sem, 16)
            g.wait_ge(b_sem, 16)
            g.collective_compute(
                "AllGather",
                mybir.AluOpType.bypass,
                replica_groups=[list(range(N_CORES))],
                ins=[cc_in.ap().opt()],
                outs=[ent_full.ap().opt()],
            ).then_inc(cc_sem)
            g.wait_ge(cc_sem, 1)
            g.wait_ge(i_sem, 32)
            g.wait_ge(r_sem, 64)
            for s in range(n_super):
                if s >= n_bufs:
                    g.wait_ge(v_sem, s - n_bufs + 1)
                for j in range(k):
                    col = s * k + j
                    q = j % n_queues
                    _set_queue(g.indirect_dma_start(
                        out=bufsl(h16_buf, s, j), out_offset=None, in_=ent_full[:],
                        in_offset=bass.IndirectOffsetOnAxis(
                            ap=h_idx[:, col:col + 1], axis=0),
                    ), q).then_inc(gh_sem, 16)
                    _set_queue(g.indirect_dma_start(
                        out=bufsl(t16_buf, s, j), out_offset=None, in_=ent_full[:],
                        in_offset=bass.IndirectOffsetOnAxis(
                            ap=t_idx[:, col:col + 1], axis=0),
                    ), q).then_inc(gt_sem, 16)
                g.dma_gather(
                    out_ap=bufsl(r_buf, s).rearrange("p (c d) -> p c d", d=D),
                    in_ap=rel,
                    idxs_ap=r_idx[:, s * 8 * k:(s + 1) * 8 * k],
                    num_idxs=n_idx,
                    num_idxs_reg=n_idx,
                    elem_size=D,
                ).then_inc(gr_sem, 16)
                for q in range(n_queues):
                    _set_queue(
                        g.dma_start(out=flush_b[:, q:q + 1],
                                    in_=flush_a[:, q:q + 1]),
                        q,
                    ).then_inc(f_sem, 16)

        @block.scalar
        def _(a):
            for s in range(n_super):
                if s >= n_bufs:
                    a.wait_ge(v_sem, s - n_bufs + 1)
                a.wait_ge(f_sem, 16 * n_queues * (s + 1))
                a.copy(out=bufsl(h_buf, s), in_=bufsl(h16_buf, s)).then_inc(c_sem, 1)
                a.copy(out=bufsl(t_buf, s), in_=bufsl(t16_buf, s)).then_inc(c_sem, 1)
            a.wait_ge(v_sem, n_super)
            a.activation(
                out=sig[:], in_=scores[:],
                func=mybir.ActivationFunctionType.Sigmoid,
            ).then_inc(s_sem, 1)

        @block.vector
        def _(v):
            for s in range(n_super):
                ksl = slice(s * k, (s + 1) * k)
                h_sl, t_sl, r_sl = bufsl(h_buf, s), bufsl(t_buf, s), bufsl(r_buf, s)
                v.wait_ge(c_sem, 2 * (s + 1))
                v.tensor_mul(out=h_sl, in0=h_sl, in1=t_sl)
                v.tensor_mul(out=h_sl, in0=h_sl, in1=r_sl)
                v.tensor_reduce(
                    out=scores[:, ksl],
                    in_=h_sl.rearrange("p (k d) -> p k d", d=D),
                    axis=mybir.AxisListType.X,
                    op=mybir.AluOpType.add,
                ).then_inc(v_sem, 1)

    nc.compile()
    return nc


def _wrap_r16(r2d, k=K):
    """[P, cols] ints -> [16, cols*8] int16 dma_gather index layout.

    Super-tile s, gather list position j = c*128 + p <-> triple (p, s*k+c);
    int16 value sits at [j % 16, s*8*k + j//16]; the 16-row pattern is
    replicated to 128 partitions on device.
    """
    p_, cols = r2d.shape
    assert p_ == P and cols % k == 0
    out = np.empty((16, cols * 8), np.int16)
    for s in range(cols // k):
        blk = r2d[:, s * k:(s + 1) * k]
        lst = blk.T.reshape(-1)
        out[:, s * 8 * k:(s + 1) * 8 * k] = lst.astype(np.int16).reshape(-1, 16).T
    return out


def _get_nc(_ent_emb=None):
    if _MODE_SHARD not in _CACHED_NC:
        _CACHED_NC[_MODE_SHARD] = _build_nc(_MODE_SHARD)
    return _CACHED_NC[_MODE_SHARD]


def make_in_maps(batch_h, batch_t, batch_r, ent_emb, rel_emb):
    global _MODE_SHARD
    bh = np.asarray(batch_h).astype(np.int32).reshape(B)
    bt = np.asarray(batch_t).astype(np.int32).reshape(B)
    br = np.asarray(batch_r).astype(np.int32).reshape(B)
    ent32 = np.asarray(ent_emb, dtype=np.float32)
    tdt = np.float16 if TABLE_FP16 else np.float32

    # compact the table to the rows actually referenced (fewer tunnel bytes);
    # fall back to the full table if they exceed the compiled capacity
    used = np.zeros(ent32.shape[0], bool)
    used[bh] = True
    used[bt] = True
    ids = np.nonzero(used)[0]
    if len(ids) <= N_CORES * SHARD_COMPACT:
        _MODE_SHARD = SHARD_COMPACT
        lut = np.empty(ent32.shape[0], np.int32)
        lut[ids] = np.arange(len(ids), dtype=np.int32)
        bh = lut[bh]
        bt = lut[bt]
        ent = np.zeros((N_CORES * SHARD_COMPACT, D), tdt)
        ent[:len(ids)] = ent32[ids].astype(tdt)
    else:
        _MODE_SHARD = SHARD_FULL
        ent = np.zeros((N_CORES * SHARD_FULL, D), tdt)
        ent[:ent32.shape[0]] = ent32.astype(tdt)
    shard = _MODE_SHARD

    rel = np.ascontiguousarray(np.asarray(rel_emb, dtype=np.float32))
    rel_w = rel.reshape(-1).view(np.int32)
    in_maps = []
    for c in range(N_CORES):
        sl = slice(c * B_CORE, (c + 1) * B_CORE)
        r16 = _wrap_r16(br[sl].reshape(P, COLS))
        aux = np.concatenate([
            bh[sl], bt[sl], r16.reshape(-1).view(np.int32), rel_w,
        ])
        assert aux.shape == (AUX_WORDS,)
        in_maps.append({
            "aux": aux,
            "ent_shard": ent[c * shard:(c + 1) * shard],
        })
    return in_maps


def kernel(batch_h, batch_t, batch_r, ent_emb, rel_emb, **_):
    in_maps = make_in_maps(batch_h, batch_t, batch_r, ent_emb, rel_emb)
    nc = _get_nc()
    res = None
    last_err = None
    for _attempt in range(3):
        try:
            res = run_bass_kernel_spmd(nc, in_maps, list(range(N_CORES)))
            break
        except Exception as e:  # transient NRT device resets on first load
            last_err = e
    if res is None:
        raise last_err
    return np.concatenate(
        [res.results[c]["score"].reshape(B_CORE) for c in range(N_CORES)]
    )
